# revision 1
# baseline (speedup 1.0000x reference)
"""Trainium2 Bass kernel for the DSAATSP dense-transformer model.

Strategy: data-parallel over batch B=8 across the 8 NeuronCores (one batch
element per core, SPMD, no collectives).  All layout prep (transposes,
fp16 casts, bias reshapes) happens on the host; on-chip the whole model is
expressed as PE matmuls + ACT activations + DVE elementwise with no
on-chip transposes:

  QT = Wq @ X^T          (per-head slices give q^T with d on partitions)
  KT = Wk @ X^T
  V  = X @ Wv^T          (tokens on partitions; a ones-column is appended
                          per head so the attn@V matmul also produces the
                          softmax row-sums for free)
  S^T = K_h @ Q_h^T      (keys on partitions -> exp(S/8) feeds attn@V
                          directly; no max-subtraction needed since the
                          logits are bounded)
  O^T,r = [V_h|1]^T @ exp(S^T/8) ;  OC = O^T / broadcast(r)
  MH^T = Wc @ OC^T + (t_emb + bc)  (t_emb computed on-device via ACT Sin)
  SC   = MH @ X^T ; out = sigmoid(+-(10a*tanh(SC/32) + c*xt + d))

where a = w00-w10, c = w01-w11, d = b0-b1 come from the 1x1 conv, since
softmax over 2 channels collapses to a sigmoid of the channel difference.

Schedule: the S^T -> exp -> attn@V chain is ACT-bound (exp), so the PE
gaps inside each head-pair's attention are filled with the NEXT pair's
Q/K projection matmuls (pr0's projections run inside the V-projection
phase).  The softmax normalization avoids PE/PSUM entirely: the row-sum
row is partition-broadcast with a stride-0 DMA and a single DVE divide
produces the normalized, fp16 attention output.  QT/KT/pt are fp32r so
the S^T and attn@V matmuls are self-loading (no InstLdweights).
"""

import math

import numpy as np

import concourse.bass as bass
import concourse.mybir as mybir
from concourse import library_config
from concourse.tile import TileContext

P = 128
NT = 1024  # node_cnt
E = 1024  # embedding dim
E2 = 512
H = 16
D = 64
HD = H * D
C = NT // P  # 8 chunks of 128
B = 8

F32 = mybir.dt.float32
F16 = mybir.dt.float16
F32R = mybir.dt.float32r
AF = mybir.ActivationFunctionType
ALU = mybir.AluOpType

# walrus in this toolchain rejects instructions with more than a few sync
# waits; hoist extras onto preceding NoOps on the same engine.
_MAX_WAITS = 1


def _split_excess_waits(nc):
    n_split = 0
    for fn in nc.m.functions:
        for bb in fn.blocks:
            new_insts = []
            for inst in bb.instructions:
                si = inst.sync_info
                if si is not None and len(si.on_wait) > _MAX_WAITS:
                    waits = list(si.on_wait)
                    k = 0
                    while len(waits) - k > _MAX_WAITS:
                        chunk = waits[k : k + _MAX_WAITS]
                        nop = mybir.InstNoOp(
                            name=f"{inst.name}-wsplit{k}",
                            engine=inst.engine,
                            ins=[],
                            outs=[],
                            sync_info=mybir.SyncInfo(on_wait=chunk, on_update=[]),
                        )
                        new_insts.append(nop)
                        k += _MAX_WAITS
                        n_split += 1
                    inst.sync_info = mybir.SyncInfo(
                        on_wait=waits[k:], on_update=list(si.on_update)
                    )
                new_insts.append(inst)
            bb.instructions[:] = new_insts
    return n_split


def build_program(bench_iters=1, stop_after=None):
    nc = bass.Bass()
    dp = nc.declare_dram_parameter
    xT_d = dp("xT", [E, NT], F16, isOutput=False)  # encoded_jobs[b].T
    xt_d = dp("xt", [NT, NT], F16, isOutput=False)
    wqT_d = dp("wqT", [E, HD], F16, isOutput=False)
    wkT_d = dp("wkT", [E, HD], F16, isOutput=False)
    wvT_d = dp("wvT", [E, HD], F16, isOutput=False)
    wcT_d = dp("wcT", [HD, E], F16, isOutput=False)
    tw1T_d = dp("tw1T", [E, E2], F16, isOutput=False)
    tw2T_d = dp("tw2T", [E2, E], F16, isOutput=False)
    tb1_d = dp("tb1", [P, 4], F32, isOutput=False)
    tb2_d = dp("tb2", [P, C], F32, isOutput=False)
    bc_d = dp("bc", [P, C], F32, isOutput=False)
    cw_d = dp("cw", [1, 4], F32, isOutput=False)
    cb_d = dp("cb", [1, 2], F32, isOutput=False)
    t_d = dp("t", [1, 1], F16, isOutput=False)
    fr_d = dp("freqs", [1, E2], F16, isOutput=False)
    out_d = dp("out", [NT, 2 * NT], F16, isOutput=True)

    import contextlib

    with TileContext(nc) as tc:
        with (
            tc.For_i(0, bench_iters, 1)
            if bench_iters > 1
            else contextlib.nullcontext()
        ):
            _build_body(nc, tc, locals(), stop_after)
    return nc


def _build_body(nc, tc, dram, stop_after=None):
    xT_d = dram["xT_d"]
    xt_d = dram["xt_d"]
    wqT_d = dram["wqT_d"]
    wkT_d = dram["wkT_d"]
    wvT_d = dram["wvT_d"]
    wcT_d = dram["wcT_d"]
    tw1T_d = dram["tw1T_d"]
    tw2T_d = dram["tw2T_d"]
    tb1_d = dram["tb1_d"]
    tb2_d = dram["tb2_d"]
    bc_d = dram["bc_d"]
    cw_d = dram["cw_d"]
    cb_d = dram["cb_d"]
    t_d = dram["t_d"]
    fr_d = dram["fr_d"]
    out_d = dram["out_d"]

    with tc.tile_pool(name="pers", bufs=1) as pers:
        XT = [pers.tile([P, NT], F16, name=f"XT{c}", tag=f"XT{c}") for c in range(C)]
        VS = [pers.tile([P, 65 * H], F32R, name=f"VS{c}", tag=f"VS{c}") for c in range(C)]
        OC = [pers.tile([P, NT], F16, name=f"OC{c}", tag=f"OC{c}") for c in range(C)]
        MHT = [pers.tile([P, NT], F16, name=f"MHT{c}", tag=f"MHT{c}") for c in range(C)]
        wcs = [pers.tile([P, E], F16, name=f"wc{c}", tag=f"wc{c}") for c in range(C)]
        acd = pers.tile([P, 3], F32, tag="acd")  # [10a, c, d] per-partition
        te = pers.tile([P, C], F32, tag="te")  # t_emb + tb2 + bc, chunk cols

        with (
            tc.tile_pool(name="wqk", bufs=16) as wqkpool,
            tc.tile_pool(name="qkt", bufs=6) as qktpool,
            tc.tile_pool(name="proj_ps", bufs=2, space="PSUM") as pps,
        ):
            # ---- projection helpers -------------------------------------
            QT = {}
            KT = {}
            wqs, wks = [], []

            def proj_steps(pr):
                """Generator yielding PE matmuls (+trailing DVE copy) for the
                4 projection chains (wq/wk x query-half) of head pair pr."""
                qt = qktpool.tile([P, NT], F32R, name=f"QT{pr}", tag="qk")
                kt = qktpool.tile([P, NT], F32R, name=f"KT{pr}", tag="qk")
                QT[pr] = qt
                KT[pr] = kt
                for ws, dst in ((wqs, qt), (wks, kt)):
                    for qh in range(2):
                        ps = pps.tile([P, 512], F32, name="pp", tag="pp")
                        for ec in range(C):
                            yield lambda ps=ps, ec=ec, ws=ws, qh=qh: nc.tensor.matmul(
                                ps[:],
                                lhsT=ws[ec][:, pr * P : (pr + 1) * P],
                                rhs=XT[ec][:, qh * 512 : (qh + 1) * 512],
                                start=(ec == 0),
                                stop=(ec == C - 1),
                            )
                        yield lambda ps=ps, dst=dst, qh=qh: nc.vector.tensor_copy(
                            dst[:, qh * 512 : (qh + 1) * 512], ps[:]
                        )

            def drain(g, n=None):
                if g is None:
                    return
                k = 0
                for step in g:
                    step()
                    k += 1
                    if n is not None and k >= n:
                        return

            with (
                tc.tile_pool(name="setup_sb", bufs=1) as ssb,
                tc.tile_pool(name="setup_ps", bufs=2, space="PSUM") as sps,
                tc.tile_pool(name="wts", bufs=8) as wvpool,
                tc.tile_pool(name="vproj_ps", bufs=2, space="PSUM") as vps,
            ):
                # ---- small setup DMAs first so the tiny setup compute is
                # not queued behind the megabyte-scale weight loads ----
                frq = ssb.tile([1, E2], F16, tag="frq")
                tsb = ssb.tile([1, 1], F16, tag="tsb")
                tb1s = ssb.tile([P, 4], F32, tag="tb1s")
                tbc = ssb.tile([P, C], F32, tag="tbc")
                bcs = ssb.tile([P, C], F32, tag="bcs")
                cwp = ssb.tile([1, 4], F32, tag="cwp")
                cbp = ssb.tile([1, 2], F32, tag="cbp")
                nc.sync.dma_start(out=frq[:], in_=fr_d[:])
                nc.sync.dma_start(out=tsb[:], in_=t_d[:])
                nc.sync.dma_start(out=tb1s[:], in_=tb1_d[:])
                nc.sync.dma_start(out=tbc[:], in_=tb2_d[:])
                nc.sync.dma_start(out=bcs[:], in_=bc_d[:])
                nc.sync.dma_start(out=cwp[:], in_=cw_d[:])
                nc.sync.dma_start(out=cbp[:], in_=cb_d[:])

                # main input loads, interleaved so V chains start early
                wvs = []
                for ec in range(C):
                    nc.sync.dma_start(
                        out=XT[ec][:], in_=xT_d[ec * P : (ec + 1) * P, :]
                    )
                    w = wvpool.tile([P, HD], F16, name=f"wv{ec}", tag="wt")
                    nc.sync.dma_start(out=w[:], in_=wvT_d[ec * P : (ec + 1) * P, :])
                    wvs.append(w)
                for ec in range(C):
                    w = wqkpool.tile([P, HD], F16, name=f"wq{ec}", tag="wqk")
                    nc.sync.dma_start(out=w[:], in_=wqT_d[ec * P : (ec + 1) * P, :])
                    wqs.append(w)
                for ec in range(C):
                    w = wqkpool.tile([P, HD], F16, name=f"wk{ec}", tag="wqk")
                    nc.sync.dma_start(out=w[:], in_=wkT_d[ec * P : (ec + 1) * P, :])
                    wks.append(w)
                tw1s = [
                    ssb.tile([P, E2], F16, name=f"tw1_{c}", tag=f"tw1_{c}")
                    for c in range(C)
                ]
                for c in range(C):
                    nc.sync.dma_start(
                        out=tw1s[c][:], in_=tw1T_d[c * P : (c + 1) * P, :]
                    )
                tw2s = [
                    ssb.tile([P, E], F16, name=f"tw2_{c}", tag=f"tw2_{c}")
                    for c in range(4)
                ]
                for c in range(4):
                    nc.sync.dma_start(
                        out=tw2s[c][:], in_=tw2T_d[c * P : (c + 1) * P, :]
                    )
                # wcs are only needed at the combine phase - load them last
                for c in range(C):
                    nc.sync.dma_start(out=wcs[c][:], in_=wcT_d[c * P : (c + 1) * P, :])

                # ---- setup compute: t_emb MLP + conv scalars (all tiny) ----
                acd_row = ssb.tile([1, 3], F16, tag="acd_row")
                ones_r = ssb.tile([1, P], F16, tag="ones_r")
                emb = ssb.tile([P, C], F16, tag="emb")
                h1 = ssb.tile([P, 4], F16, tag="h1")
                nc.vector.memset(ones_r[:], 1.0)
                pihalf = ssb.tile([P, 1], F32, tag="pihalf")
                nc.vector.memset(pihalf[:], math.pi / 2.0)
                nc.vector.tensor_add(tbc[:], tbc[:], bcs[:])

                # emb = [cos(t*freqs) | sin(t*freqs)] as a column over chunks
                for c4 in range(4):
                    aps = sps.tile([P, 1], F32, tag="sps")
                    nc.tensor.matmul(
                        aps[:],
                        lhsT=frq[0:1, c4 * P : (c4 + 1) * P],
                        rhs=tsb[0:1, 0:1],
                        start=True,
                        stop=True,
                    )
                    nc.scalar.activation(
                        emb[:, c4 : c4 + 1], aps[:], AF.Sin, bias=pihalf[:, 0:1]
                    )
                    nc.scalar.activation(emb[:, 4 + c4 : 5 + c4], aps[:], AF.Sin)

                for hc in range(4):
                    ps = sps.tile([P, 1], F32, tag="sps")
                    for ec in range(C):
                        nc.tensor.matmul(
                            ps[:],
                            lhsT=tw1s[ec][:, hc * P : (hc + 1) * P],
                            rhs=emb[:, ec : ec + 1],
                            start=(ec == 0),
                            stop=(ec == C - 1),
                        )
                    nc.scalar.activation(
                        h1[:, hc : hc + 1], ps[:], AF.Relu, bias=tb1s[:, hc : hc + 1]
                    )

                for Ec in range(C):
                    ps = sps.tile([P, 1], F32, tag="sps")
                    for hc in range(4):
                        nc.tensor.matmul(
                            ps[:],
                            lhsT=tw2s[hc][:, Ec * P : (Ec + 1) * P],
                            rhs=h1[:, hc : hc + 1],
                            start=(hc == 0),
                            stop=(hc == 3),
                        )
                    nc.scalar.activation(
                        te[:, Ec : Ec + 1], ps[:], AF.Identity, bias=tbc[:, Ec : Ec + 1]
                    )

                # acd = [10*(w00-w10), w01-w11, b0-b1] broadcast everywhere
                nc.vector.tensor_scalar(
                    acd_row[0:1, 0:1],
                    cwp[0:1, 0:1],
                    cwp[0:1, 2:3],
                    10.0,
                    ALU.subtract,
                    ALU.mult,
                )
                nc.vector.tensor_scalar(
                    acd_row[0:1, 1:2], cwp[0:1, 1:2], cwp[0:1, 3:4], None, ALU.subtract
                )
                nc.vector.tensor_scalar(
                    acd_row[0:1, 2:3], cbp[0:1, 0:1], cbp[0:1, 1:2], None, ALU.subtract
                )
                acdp = sps.tile([P, 3], F32, tag="sps")
                nc.tensor.matmul(
                    acdp[:], lhsT=ones_r[0:1, :], rhs=acd_row[0:1, :],
                    start=True, stop=True,
                )
                nc.vector.tensor_copy(acd[:], acdp[:])

                # ---- V projection (+ pr0 QK projections interleaved) ----
                onesf = ssb.tile([P, 1], F32, tag="onesf")
                nc.vector.memset(onesf[:], 1.0)
                p0 = proj_steps(0)
                for tchunk in range(C):
                    v3 = VS[tchunk].rearrange("p (h x) -> p h x", x=65)
                    nc.vector.tensor_copy(
                        v3[:, :, 64:65], onesf[:].to_broadcast((P, H, 1))
                    )
                    for ht in range(2):
                        ps = vps.tile([P, 512], F32, tag="vp")
                        for ec in range(C):
                            nc.tensor.matmul(
                                ps[:],
                                lhsT=XT[ec][:, tchunk * P : (tchunk + 1) * P],
                                rhs=wvs[ec][:, ht * 512 : (ht + 1) * 512],
                                start=(ec == 0),
                                stop=(ec == C - 1),
                            )
                        nc.vector.tensor_copy(
                            v3[:, ht * 8 : (ht + 1) * 8, 0:64],
                            ps[:].rearrange("p (h x) -> p h x", x=64),
                        )
                    drain(p0, 5)
                drain(p0)

            if stop_after == "qkv":
                for pr in range(1, C):
                    drain(proj_steps(pr))
                return

            # ---- attention: per head pair, 32 micro-steps of
            # S^T -> exp -> attn@V, with next pair's projections filling the
            # PE gaps; normalization via stride-0 DMA broadcast + DVE divide.
            with (
                tc.tile_pool(name="attn_sb", bufs=1) as asb,
                tc.tile_pool(name="sp_ps", bufs=2, space="PSUM") as spp,
                tc.tile_pool(name="ov_ps", bufs=2, space="PSUM") as ovp,
                tc.tile_pool(name="pt_sb", bufs=4) as ptp,
                tc.tile_pool(name="rb_sb", bufs=4) as rbp,
                tc.tile_pool(name="stg_sb", bufs=2) as stgp,
            ):
                for pr in range(C):
                    nxt = proj_steps(pr + 1) if pr + 1 < C else None
                    hA, hB = 2 * pr, 2 * pr + 1
                    ovs = {}
                    steps = [
                        (head, kc, qh)
                        for head in range(2)
                        for kc in range(C)
                        for qh in range(2)
                    ]
                    pts = [None] * len(steps)
                    sps_t = [None] * len(steps)

                    def do_st(i):
                        head, kc, qh = steps[i]
                        if kc == 0 and qh == 0:
                            ovs[head] = ovp.tile([D + 1, NT], F32, name=f"ov{head}", tag="ov")
                        sp = spp.tile([P, 512], F32, name="sp", tag="sp")
                        sps_t[i] = sp
                        h0 = head * D
                        nc.tensor.matmul(
                            sp[:],
                            lhsT=KT[pr][h0 : h0 + D, kc * P : (kc + 1) * P],
                            rhs=QT[pr][h0 : h0 + D, qh * 512 : (qh + 1) * 512],
                            start=True,
                            stop=True,
                        )

                    def do_exp(i):
                        pt = ptp.tile([P, 512], F32R, name="pt", tag="pt")
                        pts[i] = pt
                        nc.scalar.activation(pt[:], sps_t[i][:], AF.Exp, scale=0.125)

                    def do_pv(i):
                        head, kc, qh = steps[i]
                        h = hA if head == 0 else hB
                        nc.tensor.matmul(
                            ovs[head][:, qh * 512 : (qh + 1) * 512],
                            lhsT=VS[kc][:, 65 * h : 65 * h + 65],
                            rhs=pts[i][:],
                            start=(kc == 0),
                            stop=(kc == C - 1),
                        )

                    def do_tail(head):
                        ov = ovs[head]
                        rrow = rbp.tile([1, NT], F32, name="rrow", tag="rrow")
                        nc.vector.reciprocal(rrow[:], ov[D : D + 1, :])
                        rb = rbp.tile([D, NT], F32, name="rb", tag="rb")
                        rrow3 = rrow[0:1, :].rearrange("p (x n) -> p x n", x=1)
                        nc.sync.dma_start(
                            out=rb[:], in_=rrow3.to_broadcast((1, D, NT))
                        )
                        if head == 0:
                            nc.vector.tensor_mul(
                                OC[pr][0:D, :], ov[0:D, :], rb[:]
                            )
                        else:
                            stg = stgp.tile([D, NT], F16, name="stg", tag="stg")
                            nc.vector.tensor_mul(
                                stg[:], ov[0:D, :], rb[:]
                            )
                            nc.sync.dma_start(out=OC[pr][D : 2 * D, :], in_=stg[:])

                    do_st(0)
                    for i in range(len(steps)):
                        do_exp(i)
                        if i + 1 < len(steps):
                            do_st(i + 1)
                        drain(nxt, 1)
                        do_pv(i)
                        if i == 15:
                            do_tail(0)
                    do_tail(1)
                    drain(nxt)

            if stop_after == "attn":
                return
            # ---- multi-head combine: MH^T = Wc @ OC^T + te ----
            with (
                tc.tile_pool(name="cmb_ps", bufs=4, space="PSUM") as cps,
            ):
                for Ec in range(C):
                    for qh in range(2):
                        ps = cps.tile([P, 512], F32, tag="mm")
                        for hdc in range(C):
                            nc.tensor.matmul(
                                ps[:],
                                lhsT=wcs[hdc][:, Ec * P : (Ec + 1) * P],
                                rhs=OC[hdc][:, qh * 512 : (qh + 1) * 512],
                                start=(hdc == 0),
                                stop=(hdc == C - 1),
                            )
                        nc.vector.tensor_scalar(
                            MHT[Ec][:, qh * 512 : (qh + 1) * 512],
                            ps[:],
                            te[:, Ec : Ec + 1],
                            None,
                            ALU.add,
                        )

            if stop_after == "cmb":
                return
            # ---- final score + conv/softmax epilogue ----
            with (
                tc.tile_pool(name="fin_sb", bufs=2) as fsb,
                tc.tile_pool(name="xt_sb", bufs=3) as xsb,
                tc.tile_pool(name="fin_ps", bufs=2, space="PSUM") as fps,
            ):
                for nch in range(C):
                    xt_t = xsb.tile([P, NT], F16, tag="xtt")
                    nc.sync.dma_start(
                        out=xt_t[:], in_=xt_d[nch * P : (nch + 1) * P, :]
                    )
                    scp = fps.tile([P, NT], F32, tag="sc")
                    for mt in range(2):
                        for ec in range(C):
                            nc.tensor.matmul(
                                scp[:, mt * 512 : (mt + 1) * 512],
                                lhsT=MHT[ec][:, nch * P : (nch + 1) * P],
                                rhs=XT[ec][:, mt * 512 : (mt + 1) * 512],
                                start=(ec == 0),
                                stop=(ec == C - 1),
                            )
                    th = fsb.tile([P, NT], F16, tag="th")
                    nc.scalar.activation(th[:], scp[:], AF.Tanh, scale=1.0 / 32.0)
                    w_t = fsb.tile([P, NT], F16, tag="wt2")
                    nc.vector.tensor_scalar(
                        w_t[:], xt_t[:], acd[:, 1:2], acd[:, 2:3], ALU.mult, ALU.add
                    )
                    th2 = fsb.tile([P, NT], F16, tag="th2")
                    nc.vector.scalar_tensor_tensor(
                        th2[:], th[:], acd[:, 0:1], w_t[:], ALU.mult, ALU.add
                    )
                    ot = fsb.tile([P, 2 * NT], F16, tag="ot")
                    o3 = ot.rearrange("p (m c) -> p m c", c=2)
                    nc.scalar.activation(o3[:, :, 0], th2[:], AF.Sigmoid)
                    nc.scalar.activation(o3[:, :, 1], th2[:], AF.Sigmoid, scale=-1.0)
                    nc.sync.dma_start(
                        out=out_d[nch * P : (nch + 1) * P, :], in_=ot[:]
                    )


def make_in_maps(inputs):
    f16 = lambda a: np.ascontiguousarray(a, dtype=np.float16)
    f32 = lambda a: np.ascontiguousarray(a, dtype=np.float32)
    t = np.asarray(inputs["t"], np.float32)
    X = np.asarray(inputs["encoded_jobs"], np.float32)
    xt = np.asarray(inputs["xt"], np.float32)
    freqs = np.exp(
        -math.log(10000.0) * np.arange(E2, dtype=np.float32) / float(E2)
    )
    shared = {
        "wqT": f16(np.asarray(inputs["Wq"]).T),
        "wkT": f16(np.asarray(inputs["Wk"]).T),
        "wvT": f16(np.asarray(inputs["Wv"]).T),
        "wcT": f16(np.asarray(inputs["Wc"]).T),
        "tw1T": f16(np.asarray(inputs["tW1"]).T),
        "tw2T": f16(np.asarray(inputs["tW2"]).T),
        "tb1": f32(np.asarray(inputs["tb1"]).reshape(4, P).T),
        "tb2": f32(np.asarray(inputs["tb2"]).reshape(C, P).T),
        "bc": f32(np.asarray(inputs["bc"]).reshape(C, P).T),
        "cw": f32(np.asarray(inputs["conv_w"]).reshape(1, 4)),
        "cb": f32(np.asarray(inputs["conv_b"]).reshape(1, 2)),
        "freqs": f16(freqs.reshape(1, E2)),
    }
    in_maps = []
    for b in range(B):
        m = dict(shared)
        m["xT"] = f16(X[b].T)
        m["xt"] = f16(xt[b])
        m["t"] = f16(t[b].reshape(1, 1))
        in_maps.append(m)
    return in_maps


_CACHE = {}


def _get_runner(bench_iters=1):
    """Build the SPMD executable once (same path run_bass_kernel_spmd takes
    under axon -- bass2jax custom call through PJRT on 8 cores -- but with
    the jitted executable cached so repeat calls skip recompilation)."""
    key = ("run", bench_iters)
    if key in _CACHE:
        return _CACHE[key]
    import jax
    from jax.experimental.shard_map import shard_map
    from jax.sharding import Mesh, PartitionSpec

    from concourse import bass2jax

    bass2jax.install_neuronx_cc_hook()
    nc = build_program(bench_iters)
    _split_excess_waits(nc)
    partition_name = nc.partition_id_tensor.name if nc.partition_id_tensor else None
    in_names, out_names, out_avals, zero_outs = [], [], [], []
    for alloc in nc.m.functions[0].allocations:
        if not isinstance(alloc, mybir.MemoryLocationSet):
            continue
        name = alloc.memorylocations[0].name
        if alloc.kind == "ExternalInput":
            if name != partition_name:
                in_names.append(name)
        elif alloc.kind == "ExternalOutput":
            shape = tuple(alloc.tensor_shape)
            dt = mybir.dt.np(alloc.dtype)
            out_names.append(name)
            out_avals.append(jax.core.ShapedArray(shape, dt))
            zero_outs.append(np.zeros(shape, dt))
    n_params = len(in_names)
    all_in = in_names + out_names
    if partition_name is not None:
        all_in = all_in + [partition_name]
    all_in = tuple(all_in)

    def _body(*args):
        operands = list(args)
        if partition_name is not None:
            operands.append(bass2jax.partition_id_tensor())
        outs = bass2jax._bass_exec_p.bind(
            *operands,
            out_avals=tuple(out_avals),
            in_names=all_in,
            out_names=tuple(out_names),
            lowering_input_output_aliases=(),
            sim_require_finite=True,
            sim_require_nnan=True,
            nc=nc,
        )
        return tuple(outs)

    devices = jax.devices()[:B]
    mesh = Mesh(np.asarray(devices), ("core",))
    n_outs = len(out_names)
    in_specs = (PartitionSpec("core"),) * (n_params + n_outs)
    out_specs = (PartitionSpec("core"),) * n_outs
    donate = tuple(range(n_params, n_params + n_outs))
    sharded = jax.jit(
        shard_map(
            _body, mesh=mesh, in_specs=in_specs, out_specs=out_specs, check_rep=False
        ),
        donate_argnums=donate,
        keep_unused=True,
    )
    _CACHE[key] = (sharded, in_names, out_names, out_avals, zero_outs, mesh)
    return _CACHE[key]


def _concat_inputs(in_maps, bench_iters=1):
    sharded, in_names, out_names, out_avals, zero_outs, mesh = _get_runner(bench_iters)
    concat_in = [
        np.concatenate([np.asarray(m[n]) for m in in_maps], axis=0) for n in in_names
    ]
    concat_zeros = [
        np.zeros((B * z.shape[0], *z.shape[1:]), z.dtype) for z in zero_outs
    ]
    return concat_in, concat_zeros


def _run_spmd(in_maps):
    sharded, in_names, out_names, out_avals, zero_outs, mesh = _get_runner()
    concat_in, concat_zeros = _concat_inputs(in_maps)
    out_arrs = sharded(*concat_in, *concat_zeros)
    return [
        {
            name: np.asarray(out_arrs[i]).reshape(B, *out_avals[i].shape)[c]
            for i, name in enumerate(out_names)
        }
        for c in range(B)
    ]


def _wall_times(in_maps, bench_iters, reps):
    import time

    import jax
    from jax.sharding import NamedSharding, PartitionSpec

    sharded, in_names, out_names, out_avals, zero_outs, mesh = _get_runner(bench_iters)
    concat_in, concat_zeros = _concat_inputs(in_maps, bench_iters)
    sh = NamedSharding(mesh, PartitionSpec("core"))
    dev_in = [jax.device_put(a, sh) for a in concat_in]
    jax.block_until_ready(dev_in)
    times = []
    out = None
    for _ in range(reps + 1):
        dev_z = [jax.device_put(a, sh) for a in concat_zeros]
        jax.block_until_ready(dev_z)
        t0 = time.perf_counter()
        out = sharded(*dev_in, *dev_z)
        jax.block_until_ready(out)
        times.append((time.perf_counter() - t0) * 1e9)
    return times[1:], out


def bench(in_maps, lo=129, hi=2049, reps=7):
    """Device-side loop timing: the kernel body repeats inside one NEFF via
    For_i; per-iteration time is the slope between two large loop counts,
    which cancels the (noisy) axon RPC overhead."""
    import time

    import jax
    from jax.sharding import NamedSharding, PartitionSpec

    runs = {}
    for it in (lo, hi):
        sharded, in_names, out_names, out_avals, zero_outs, mesh = _get_runner(it)
        ci, cz = _concat_inputs(in_maps, it)
        sh = NamedSharding(mesh, PartitionSpec("core"))
        dev_in = [jax.device_put(a, sh) for a in ci]
        jax.block_until_ready(dev_in)
        runs[it] = (sharded, dev_in, cz, sh)
    times = {lo: [], hi: []}
    for r in range(reps + 1):
        for it in (lo, hi):
            sharded, dev_in, cz, sh = runs[it]
            dev_z = [jax.device_put(a, sh) for a in cz]
            jax.block_until_ready(dev_z)
            t0 = time.perf_counter()
            out = sharded(*dev_in, *dev_z)
            jax.block_until_ready(out)
            if r > 0:
                times[it].append(time.perf_counter() - t0)
    per_iter = (min(times[hi]) - min(times[lo])) / (hi - lo) * 1e9
    return per_iter, (min(times[lo]) * 1e9, min(times[hi]) * 1e9)


def kernel(**inputs):
    results = _run_spmd(make_in_maps(inputs))
    out = np.stack([r["out"].reshape(NT, NT, 2) for r in results])
    return out.astype(np.float32)



# revision 2
# speedup vs baseline: 1.2489x; 1.2489x over previous
"""Trainium2 Bass kernel for the DSAATSP dense-transformer model (v2).

Strategy: data-parallel over batch B=8 across the 8 NeuronCores (one batch
element per core, SPMD, no collectives).  Relative to v1 the heavy GEMMs
move to fp8e4 with perf_mode=DoubleRow (2 k-tiles of 128 per instruction,
measured 310ns per 256-deep x 512-col matmul vs 241ns for a 128-deep f16
one => 1.55x), and the attention S^T matmuls move from fp32r (measured
454ns) to f16 (241ns):

  q16^T/k16^T = (16Wq)8 @ X8^T   fp8 DoubleRow chains -> f16 QT/KT
  v16         = X8 @ (16Wv)8^T   fp8 DoubleRow -> fp8 VSp (+exact ones col)
  S^T  = K16_h @ Q16_h^T         f16, 64-deep, psum = 256*S_true
  pt   = exp(S^T * 0.125/256)    one ACT op per key-chunk [128,1024],
                                 written fp8 directly in the DoubleRow
                                 kt-interleaved pair layout
  O^T,r = [v16|1]^T @ pt         fp8 DoubleRow over key pairs
  OC8  = (O^T * 4) / r           = 64*OC_true, fp8, DVE scalar_tensor_tensor
  MH^T = (16Wc)8 @ OC8^T / 1024 + t_emb + bc   fp8 DoubleRow + DVE epilogue
  SC   = MH @ X^T (f16) ; out = sigmoid(+-(10a*tanh(SC/32) + c*xt + d))

where a = w00-w10, c = w01-w11, d = b0-b1 come from the 1x1 conv, since
softmax over 2 channels collapses to a sigmoid of the channel difference.

All fp8 operands are pre-scaled by 16 on the host (keeps values in e4m3's
sweet spot); the factors cancel exactly via the exp scale (1/256), the
attention-tail *4 and the combine 1/1024.

Schedule: per head, 8 S^T+exp steps stream into 4 fp8 pt pair-tiles, then
2 DoubleRow PV chains (one per query half) + normalization tail; the next
pair's Q/K projection chains drain into the PE gaps (ACT exp is the
binding engine during attention).  PSUM: sp 2x2 banks + ov 2x1 + proj 2x1
= 8 banks exactly.
"""

import math

import numpy as np

import concourse.bass as bass
import concourse.mybir as mybir
from concourse import library_config
from concourse.tile import TileContext

P = 128
NT = 1024  # node_cnt
E = 1024  # embedding dim
E2 = 512
H = 16
D = 64
HD = H * D
C = NT // P  # 8 chunks of 128
C2 = 4  # 4 chunks of 256 (DoubleRow k-tile pairs)
B = 8

F32 = mybir.dt.float32
F16 = mybir.dt.float16
F8 = mybir.dt.float8e4
AF = mybir.ActivationFunctionType
ALU = mybir.AluOpType
DR = mybir.MatmulPerfMode.DoubleRow

SW = 16.0  # host-side fp8 operand scale
EXP_SCALE = 0.125 / (SW * SW)  # exp(S_true/8) from the 256x-scaled psum

# walrus in this toolchain rejects instructions with more than a few sync
# waits; hoist extras onto preceding NoOps on the same engine.
_MAX_WAITS = 1


def _split_excess_waits(nc):
    n_split = 0
    for fn in nc.m.functions:
        for bb in fn.blocks:
            new_insts = []
            for inst in bb.instructions:
                si = inst.sync_info
                if si is not None and len(si.on_wait) > _MAX_WAITS:
                    waits = list(si.on_wait)
                    k = 0
                    while len(waits) - k > _MAX_WAITS:
                        chunk = waits[k : k + _MAX_WAITS]
                        nop = mybir.InstNoOp(
                            name=f"{inst.name}-wsplit{k}",
                            engine=inst.engine,
                            ins=[],
                            outs=[],
                            sync_info=mybir.SyncInfo(on_wait=chunk, on_update=[]),
                        )
                        new_insts.append(nop)
                        k += _MAX_WAITS
                        n_split += 1
                    inst.sync_info = mybir.SyncInfo(
                        on_wait=waits[k:], on_update=list(si.on_update)
                    )
                new_insts.append(inst)
            bb.instructions[:] = new_insts
    return n_split


def build_program(bench_iters=1, stop_after=None):
    nc = bass.Bass()
    dp = nc.declare_dram_parameter
    xT8_d = dp("xT8", [P, 8 * NT], F8, isOutput=False)  # (c2,kt)-interleaved X^T
    wq8_d = dp("wq8", [P, 8 * HD], F8, isOutput=False)
    wk8_d = dp("wk8", [P, 8 * HD], F8, isOutput=False)
    wv8_d = dp("wv8", [P, 8 * HD], F8, isOutput=False)
    wc8_d = dp("wc8", [P, 8 * E], F8, isOutput=False)
    xT_d = dp("xT", [E, NT], F16, isOutput=False)  # encoded_jobs[b].T
    xt_d = dp("xt", [NT, NT], F16, isOutput=False)
    tw1T_d = dp("tw1T", [E, E2], F16, isOutput=False)
    tw2T_d = dp("tw2T", [E2, E], F16, isOutput=False)
    tb1_d = dp("tb1", [P, 4], F32, isOutput=False)
    tb2_d = dp("tb2", [P, C], F32, isOutput=False)
    bc_d = dp("bc", [P, C], F32, isOutput=False)
    cw_d = dp("cw", [1, 4], F32, isOutput=False)
    cb_d = dp("cb", [1, 2], F32, isOutput=False)
    t_d = dp("t", [1, 1], F16, isOutput=False)
    fr_d = dp("freqs", [1, E2], F16, isOutput=False)
    out_d = dp("out", [NT, 2 * NT], F16, isOutput=True)

    import contextlib

    with TileContext(nc) as tc:
        with (
            tc.For_i(0, bench_iters, 1)
            if bench_iters > 1
            else contextlib.nullcontext()
        ):
            _build_body(nc, tc, locals(), stop_after)
    return nc


def _build_body(nc, tc, dram, stop_after=None):
    xT8_d = dram["xT8_d"]
    wq8_d = dram["wq8_d"]
    wk8_d = dram["wk8_d"]
    wv8_d = dram["wv8_d"]
    wc8_d = dram["wc8_d"]
    xT_d = dram["xT_d"]
    xt_d = dram["xt_d"]
    tw1T_d = dram["tw1T_d"]
    tw2T_d = dram["tw2T_d"]
    tb1_d = dram["tb1_d"]
    tb2_d = dram["tb2_d"]
    bc_d = dram["bc_d"]
    cw_d = dram["cw_d"]
    cb_d = dram["cb_d"]
    t_d = dram["t_d"]
    fr_d = dram["fr_d"]
    out_d = dram["out_d"]

    with tc.tile_pool(name="pers", bufs=1) as pers:
        XT8 = pers.tile([P, 8 * NT], F8, tag="XT8")
        WQ8 = pers.tile([P, 8 * HD], F8, tag="WQ8")
        WK8 = pers.tile([P, 8 * HD], F8, tag="WK8")
        WV8 = pers.tile([P, 8 * HD], F8, tag="WV8")
        WC8 = pers.tile([P, 8 * E], F8, tag="WC8")
        XT = [pers.tile([P, NT], F16, name=f"XT{c}", tag=f"XT{c}") for c in range(C)]
        VSp = [
            pers.tile([P, 2 * 65 * H], F8, name=f"VS{c}", tag=f"VS{c}")
            for c in range(C2)
        ]
        OCp = [
            pers.tile([P, 2 * NT], F8, name=f"OC{c}", tag=f"OC{c}")
            for c in range(C2)
        ]
        MHT = [pers.tile([P, NT], F16, name=f"MHT{c}", tag=f"MHT{c}") for c in range(C)]
        acd = pers.tile([P, 3], F32, tag="acd")  # [10a, c, d] per-partition
        te = pers.tile([P, C], F32, tag="te")  # t_emb + tb2 + bc, chunk cols

        XT8v = XT8.rearrange("p (c k n) -> p c k n", k=2, n=NT)
        WQ8v = WQ8.rearrange("p (c k n) -> p c k n", k=2, n=HD)
        WK8v = WK8.rearrange("p (c k n) -> p c k n", k=2, n=HD)
        WV8v = WV8.rearrange("p (c k n) -> p c k n", k=2, n=HD)
        WC8v = WC8.rearrange("p (c k n) -> p c k n", k=2, n=E)

        with (
            tc.tile_pool(name="qkt", bufs=4) as qktpool,
            tc.tile_pool(name="proj_ps", bufs=2, space="PSUM") as pps,
        ):
            # ---- projection helpers -------------------------------------
            QT = {}
            KT = {}

            def proj_steps(pr):
                """Generator yielding PE matmuls (+trailing DVE copy) for the
                4 DoubleRow projection chains (wq/wk x query-half) of head
                pair pr."""
                qt = qktpool.tile([P, NT], F16, name=f"QT{pr}", tag="qk")
                kt = qktpool.tile([P, NT], F16, name=f"KT{pr}", tag="qk")
                QT[pr] = qt
                KT[pr] = kt
                for w8v, dst in ((WQ8v, qt), (WK8v, kt)):
                    for qh in range(2):
                        ps = pps.tile([P, 512], F32, name="pp", tag="pp")
                        for c2 in range(C2):
                            yield lambda ps=ps, c2=c2, w8v=w8v, qh=qh: nc.tensor.matmul(
                                ps[:],
                                lhsT=w8v[:, c2, :, pr * P : (pr + 1) * P],
                                rhs=XT8v[:, c2, :, qh * 512 : (qh + 1) * 512],
                                start=(c2 == 0),
                                stop=(c2 == C2 - 1),
                                perf_mode=DR,
                            )
                        yield lambda ps=ps, dst=dst, qh=qh: nc.vector.tensor_copy(
                            dst[:, qh * 512 : (qh + 1) * 512], ps[:]
                        )

            def drain(g, n=None):
                if g is None:
                    return
                k = 0
                for step in g:
                    step()
                    k += 1
                    if n is not None and k >= n:
                        return

            with (
                tc.tile_pool(name="setup_sb", bufs=1) as ssb,
                tc.tile_pool(name="setup_ps", bufs=2, space="PSUM") as sps,
                tc.tile_pool(name="vproj_ps", bufs=2, space="PSUM") as vps,
            ):
                # ---- small setup DMAs first so the tiny setup compute is
                # not queued behind the megabyte-scale weight loads ----
                frq = ssb.tile([1, E2], F16, tag="frq")
                tsb = ssb.tile([1, 1], F16, tag="tsb")
                tb1s = ssb.tile([P, 4], F32, tag="tb1s")
                tbc = ssb.tile([P, C], F32, tag="tbc")
                bcs = ssb.tile([P, C], F32, tag="bcs")
                cwp = ssb.tile([1, 4], F32, tag="cwp")
                cbp = ssb.tile([1, 2], F32, tag="cbp")
                nc.sync.dma_start(out=frq[:], in_=fr_d[:])
                nc.sync.dma_start(out=tsb[:], in_=t_d[:])
                nc.sync.dma_start(out=tb1s[:], in_=tb1_d[:])
                nc.sync.dma_start(out=tbc[:], in_=tb2_d[:])
                nc.sync.dma_start(out=bcs[:], in_=bc_d[:])
                nc.sync.dma_start(out=cwp[:], in_=cw_d[:])
                nc.sync.dma_start(out=cbp[:], in_=cb_d[:])

                # main input loads: V-projection inputs first
                nc.sync.dma_start(out=XT8[:], in_=xT8_d[:])
                nc.sync.dma_start(out=WV8[:], in_=wv8_d[:])
                nc.sync.dma_start(out=WQ8[:], in_=wq8_d[:])
                nc.sync.dma_start(out=WK8[:], in_=wk8_d[:])
                tw1s = [
                    ssb.tile([P, E2], F16, name=f"tw1_{c}", tag=f"tw1_{c}")
                    for c in range(C)
                ]
                for c in range(C):
                    nc.sync.dma_start(
                        out=tw1s[c][:], in_=tw1T_d[c * P : (c + 1) * P, :]
                    )
                tw2s = [
                    ssb.tile([P, E], F16, name=f"tw2_{c}", tag=f"tw2_{c}")
                    for c in range(4)
                ]
                for c in range(4):
                    nc.sync.dma_start(
                        out=tw2s[c][:], in_=tw2T_d[c * P : (c + 1) * P, :]
                    )
                # combine/final inputs are only needed later - load last
                nc.sync.dma_start(out=WC8[:], in_=wc8_d[:])
                for c in range(C):
                    nc.sync.dma_start(out=XT[c][:], in_=xT_d[c * P : (c + 1) * P, :])

                # ---- setup compute: t_emb MLP + conv scalars (all tiny) ----
                acd_row = ssb.tile([1, 3], F16, tag="acd_row")
                ones_r = ssb.tile([1, P], F16, tag="ones_r")
                emb = ssb.tile([P, C], F16, tag="emb")
                h1 = ssb.tile([P, 4], F16, tag="h1")
                nc.vector.memset(ones_r[:], 1.0)
                pihalf = ssb.tile([P, 1], F32, tag="pihalf")
                nc.vector.memset(pihalf[:], math.pi / 2.0)
                nc.vector.tensor_add(tbc[:], tbc[:], bcs[:])

                # emb = [cos(t*freqs) | sin(t*freqs)] as a column over chunks
                for c4 in range(4):
                    aps = sps.tile([P, 1], F32, tag="sps")
                    nc.tensor.matmul(
                        aps[:],
                        lhsT=frq[0:1, c4 * P : (c4 + 1) * P],
                        rhs=tsb[0:1, 0:1],
                        start=True,
                        stop=True,
                    )
                    nc.scalar.activation(
                        emb[:, c4 : c4 + 1], aps[:], AF.Sin, bias=pihalf[:, 0:1]
                    )
                    nc.scalar.activation(emb[:, 4 + c4 : 5 + c4], aps[:], AF.Sin)

                for hc in range(4):
                    ps = sps.tile([P, 1], F32, tag="sps")
                    for ec in range(C):
                        nc.tensor.matmul(
                            ps[:],
                            lhsT=tw1s[ec][:, hc * P : (hc + 1) * P],
                            rhs=emb[:, ec : ec + 1],
                            start=(ec == 0),
                            stop=(ec == C - 1),
                        )
                    nc.scalar.activation(
                        h1[:, hc : hc + 1], ps[:], AF.Relu, bias=tb1s[:, hc : hc + 1]
                    )

                for Ec in range(C):
                    ps = sps.tile([P, 1], F32, tag="sps")
                    for hc in range(4):
                        nc.tensor.matmul(
                            ps[:],
                            lhsT=tw2s[hc][:, Ec * P : (Ec + 1) * P],
                            rhs=h1[:, hc : hc + 1],
                            start=(hc == 0),
                            stop=(hc == 3),
                        )
                    nc.scalar.activation(
                        te[:, Ec : Ec + 1], ps[:], AF.Identity, bias=tbc[:, Ec : Ec + 1]
                    )

                # acd = [10*(w00-w10), w01-w11, b0-b1] broadcast everywhere
                nc.vector.tensor_scalar(
                    acd_row[0:1, 0:1],
                    cwp[0:1, 0:1],
                    cwp[0:1, 2:3],
                    10.0,
                    ALU.subtract,
                    ALU.mult,
                )
                nc.vector.tensor_scalar(
                    acd_row[0:1, 1:2], cwp[0:1, 1:2], cwp[0:1, 3:4], None, ALU.subtract
                )
                nc.vector.tensor_scalar(
                    acd_row[0:1, 2:3], cbp[0:1, 0:1], cbp[0:1, 1:2], None, ALU.subtract
                )
                acdp = sps.tile([P, 3], F32, tag="sps")
                nc.tensor.matmul(
                    acdp[:], lhsT=ones_r[0:1, :], rhs=acd_row[0:1, :],
                    start=True, stop=True,
                )
                nc.vector.tensor_copy(acd[:], acdp[:])

                # ---- V projection (+ pr0 QK projections interleaved) ----
                onesf = ssb.tile([P, 1], F32, tag="onesf")
                nc.vector.memset(onesf[:], 1.0)
                p0 = proj_steps(0)
                for tc2 in range(C2):
                    v3 = VSp[tc2].rearrange("p (k h x) -> p k h x", k=2, x=65)
                    for kt in range(2):
                        tchunk = 2 * tc2 + kt
                        nc.vector.tensor_copy(
                            v3[:, kt, :, 64:65], onesf[:].to_broadcast((P, H, 1))
                        )
                        for ht in range(2):
                            ps = vps.tile([P, 512], F32, tag="vp")
                            for c2 in range(C2):
                                nc.tensor.matmul(
                                    ps[:],
                                    lhsT=XT8v[
                                        :, c2, :, tchunk * P : (tchunk + 1) * P
                                    ],
                                    rhs=WV8v[:, c2, :, ht * 512 : (ht + 1) * 512],
                                    start=(c2 == 0),
                                    stop=(c2 == C2 - 1),
                                    perf_mode=DR,
                                )
                            nc.vector.tensor_copy(
                                v3[:, kt, ht * 8 : (ht + 1) * 8, 0:64],
                                ps[:].rearrange("p (h x) -> p h x", x=64),
                            )
                        drain(p0, 3)
                drain(p0)

            if stop_after == "qkv":
                for pr in range(1, C):
                    drain(proj_steps(pr))
                return

            # ---- attention: per head, 8 S^T (f16) + exp (fp8 pair layout)
            # steps, then 2 DoubleRow PV chains + normalization tails; the
            # next pair's projections fill the PE gaps.
            with (
                tc.tile_pool(name="sp_ps", bufs=2, space="PSUM") as spp,
                tc.tile_pool(name="ov_ps", bufs=2, space="PSUM") as ovp,
                tc.tile_pool(name="pt_sb", bufs=5) as ptp,
                tc.tile_pool(name="rb_sb", bufs=4) as rbp,
                tc.tile_pool(name="stg_sb", bufs=2) as stgp,
            ):
                for pr in range(C):
                    nxt = proj_steps(pr + 1) if pr + 1 < C else None
                    c2o, kto = pr // 2, pr % 2
                    ocv = OCp[c2o].rearrange("p (k n) -> p k n", k=2)
                    for head in range(2):
                        h = 2 * pr + head
                        h0 = head * D
                        pts = [
                            ptp.tile([P, 2 * NT], F8, name=f"pt{c}", tag="pt")
                            for c in range(C2)
                        ]
                        for kc in range(C):
                            sp = spp.tile([P, NT], F32, name="sp", tag="sp")
                            for qh in range(2):
                                nc.tensor.matmul(
                                    sp[:, qh * 512 : (qh + 1) * 512],
                                    lhsT=KT[pr][
                                        h0 : h0 + D, kc * P : (kc + 1) * P
                                    ],
                                    rhs=QT[pr][
                                        h0 : h0 + D, qh * 512 : (qh + 1) * 512
                                    ],
                                    start=True,
                                    stop=True,
                                )
                            ptv = pts[kc // 2].rearrange("p (k n) -> p k n", k=2)
                            nc.scalar.activation(
                                ptv[:, kc % 2, :], sp[:], AF.Exp, scale=EXP_SCALE
                            )
                            drain(nxt, 1)
                        for qh in range(2):
                            ov = ovp.tile([D + 1, 512], F32, name="ov", tag="ov")
                            for c in range(C2):
                                ptv = pts[c].rearrange("p (k n) -> p k n", k=2)
                                nc.tensor.matmul(
                                    ov[:],
                                    lhsT=VSp[c].rearrange(
                                        "p (k n) -> p k n", k=2
                                    )[:, :, 65 * h : 65 * h + 65],
                                    rhs=ptv[:, :, qh * 512 : (qh + 1) * 512],
                                    start=(c == 0),
                                    stop=(c == C2 - 1),
                                    perf_mode=DR,
                                )
                            drain(nxt, 1)
                            # normalization tail for (head, qh)
                            rr = rbp.tile([1, 512], F32, name="rr", tag="rr")
                            nc.vector.reciprocal(rr[:], ov[D : D + 1, :])
                            rb = rbp.tile([D, 512], F32, name="rb", tag="rb")
                            rrow3 = rr[0:1, :].rearrange("p (x n) -> p x n", x=1)
                            nc.sync.dma_start(
                                out=rb[:], in_=rrow3.to_broadcast((1, D, 512))
                            )
                            if head == 0:
                                nc.vector.scalar_tensor_tensor(
                                    ocv[0:D, kto, qh * 512 : (qh + 1) * 512],
                                    ov[0:D, :],
                                    4.0,
                                    rb[:],
                                    ALU.mult,
                                    ALU.mult,
                                )
                            else:
                                stg = stgp.tile([D, 512], F8, name="stg", tag="stg")
                                nc.vector.scalar_tensor_tensor(
                                    stg[:], ov[0:D, :], 4.0, rb[:],
                                    ALU.mult, ALU.mult,
                                )
                                nc.sync.dma_start(
                                    out=ocv[D : 2 * D, kto, qh * 512 : (qh + 1) * 512],
                                    in_=stg[:],
                                )
                    drain(nxt)

            if stop_after == "attn":
                return
            # ---- multi-head combine: MH^T = (16Wc)8 @ OC8^T / 1024 + te ----
            with (
                tc.tile_pool(name="cmb_ps", bufs=4, space="PSUM") as cps,
            ):
                for Ec in range(C):
                    for qh in range(2):
                        ps = cps.tile([P, 512], F32, tag="mm")
                        for c2 in range(C2):
                            nc.tensor.matmul(
                                ps[:],
                                lhsT=WC8v[:, c2, :, Ec * P : (Ec + 1) * P],
                                rhs=OCp[c2].rearrange("p (k n) -> p k n", k=2)[
                                    :, :, qh * 512 : (qh + 1) * 512
                                ],
                                start=(c2 == 0),
                                stop=(c2 == C2 - 1),
                                perf_mode=DR,
                            )
                        nc.vector.tensor_scalar(
                            MHT[Ec][:, qh * 512 : (qh + 1) * 512],
                            ps[:],
                            1.0 / (SW * SW * 4.0),
                            te[:, Ec : Ec + 1],
                            ALU.mult,
                            ALU.add,
                        )

            if stop_after == "cmb":
                return
            # ---- final score + conv/softmax epilogue ----
            with (
                tc.tile_pool(name="fin_sb", bufs=2) as fsb,
                tc.tile_pool(name="xt_sb", bufs=3) as xsb,
                tc.tile_pool(name="fin_ps", bufs=2, space="PSUM") as fps,
            ):
                for nch in range(C):
                    xt_t = xsb.tile([P, NT], F16, tag="xtt")
                    nc.sync.dma_start(
                        out=xt_t[:], in_=xt_d[nch * P : (nch + 1) * P, :]
                    )
                    scp = fps.tile([P, NT], F32, tag="sc")
                    for mt in range(2):
                        for ec in range(C):
                            nc.tensor.matmul(
                                scp[:, mt * 512 : (mt + 1) * 512],
                                lhsT=MHT[ec][:, nch * P : (nch + 1) * P],
                                rhs=XT[ec][:, mt * 512 : (mt + 1) * 512],
                                start=(ec == 0),
                                stop=(ec == C - 1),
                            )
                    th = fsb.tile([P, NT], F16, tag="th")
                    nc.scalar.activation(th[:], scp[:], AF.Tanh, scale=1.0 / 32.0)
                    w_t = fsb.tile([P, NT], F16, tag="wt2")
                    nc.vector.tensor_scalar(
                        w_t[:], xt_t[:], acd[:, 1:2], acd[:, 2:3], ALU.mult, ALU.add
                    )
                    th2 = fsb.tile([P, NT], F16, tag="th2")
                    nc.vector.scalar_tensor_tensor(
                        th2[:], th[:], acd[:, 0:1], w_t[:], ALU.mult, ALU.add
                    )
                    ot = fsb.tile([P, 2 * NT], F16, tag="ot")
                    o3 = ot.rearrange("p (m c) -> p m c", c=2)
                    nc.scalar.activation(o3[:, :, 0], th2[:], AF.Sigmoid)
                    nc.scalar.activation(o3[:, :, 1], th2[:], AF.Sigmoid, scale=-1.0)
                    nc.sync.dma_start(
                        out=out_d[nch * P : (nch + 1) * P, :], in_=ot[:]
                    )


_F8NP = mybir.dt.np(F8)


def _dr_layout(a, scale=1.0):
    """[1024, Cc] -> [128, 8*Cc] fp8 with (c2, kt) k-tile interleaving:
    out[k, (c2*2+kt)*Cc + m] = a[c2*256 + kt*128 + k, m] * scale."""
    a = np.asarray(a, np.float32) * scale
    Cc = a.shape[1]
    return np.ascontiguousarray(
        a.reshape(4, 2, 128, Cc).transpose(2, 0, 1, 3).reshape(128, 8 * Cc)
    ).astype(_F8NP)


def make_in_maps(inputs):
    f16 = lambda a: np.ascontiguousarray(a, dtype=np.float16)
    f32 = lambda a: np.ascontiguousarray(a, dtype=np.float32)
    t = np.asarray(inputs["t"], np.float32)
    X = np.asarray(inputs["encoded_jobs"], np.float32)
    xt = np.asarray(inputs["xt"], np.float32)
    freqs = np.exp(
        -math.log(10000.0) * np.arange(E2, dtype=np.float32) / float(E2)
    )
    shared = {
        "wq8": _dr_layout(np.asarray(inputs["Wq"]).T, SW),
        "wk8": _dr_layout(np.asarray(inputs["Wk"]).T, SW),
        "wv8": _dr_layout(np.asarray(inputs["Wv"]).T, SW),
        "wc8": _dr_layout(np.asarray(inputs["Wc"]).T, SW),
        "tw1T": f16(np.asarray(inputs["tW1"]).T),
        "tw2T": f16(np.asarray(inputs["tW2"]).T),
        "tb1": f32(np.asarray(inputs["tb1"]).reshape(4, P).T),
        "tb2": f32(np.asarray(inputs["tb2"]).reshape(C, P).T),
        "bc": f32(np.asarray(inputs["bc"]).reshape(C, P).T),
        "cw": f32(np.asarray(inputs["conv_w"]).reshape(1, 4)),
        "cb": f32(np.asarray(inputs["conv_b"]).reshape(1, 2)),
        "freqs": f16(freqs.reshape(1, E2)),
    }
    in_maps = []
    for b in range(B):
        m = dict(shared)
        m["xT8"] = _dr_layout(X[b].T)
        m["xT"] = f16(X[b].T)
        m["xt"] = f16(xt[b])
        m["t"] = f16(t[b].reshape(1, 1))
        in_maps.append(m)
    return in_maps


_CACHE = {}


def _get_runner(bench_iters=1):
    """Build the SPMD executable once (same path run_bass_kernel_spmd takes
    under axon -- bass2jax custom call through PJRT on 8 cores -- but with
    the jitted executable cached so repeat calls skip recompilation)."""
    key = ("run", bench_iters)
    if key in _CACHE:
        return _CACHE[key]
    import jax
    from jax.experimental.shard_map import shard_map
    from jax.sharding import Mesh, PartitionSpec

    from concourse import bass2jax

    bass2jax.install_neuronx_cc_hook()
    nc = build_program(bench_iters)
    _split_excess_waits(nc)
    partition_name = nc.partition_id_tensor.name if nc.partition_id_tensor else None
    in_names, out_names, out_avals, zero_outs = [], [], [], []
    for alloc in nc.m.functions[0].allocations:
        if not isinstance(alloc, mybir.MemoryLocationSet):
            continue
        name = alloc.memorylocations[0].name
        if alloc.kind == "ExternalInput":
            if name != partition_name:
                in_names.append(name)
        elif alloc.kind == "ExternalOutput":
            shape = tuple(alloc.tensor_shape)
            dt = mybir.dt.np(alloc.dtype)
            out_names.append(name)
            out_avals.append(jax.core.ShapedArray(shape, dt))
            zero_outs.append(np.zeros(shape, dt))
    n_params = len(in_names)
    all_in = in_names + out_names
    if partition_name is not None:
        all_in = all_in + [partition_name]
    all_in = tuple(all_in)

    def _body(*args):
        operands = list(args)
        if partition_name is not None:
            operands.append(bass2jax.partition_id_tensor())
        outs = bass2jax._bass_exec_p.bind(
            *operands,
            out_avals=tuple(out_avals),
            in_names=all_in,
            out_names=tuple(out_names),
            lowering_input_output_aliases=(),
            sim_require_finite=True,
            sim_require_nnan=True,
            nc=nc,
        )
        return tuple(outs)

    devices = jax.devices()[:B]
    mesh = Mesh(np.asarray(devices), ("core",))
    n_outs = len(out_names)
    in_specs = (PartitionSpec("core"),) * (n_params + n_outs)
    out_specs = (PartitionSpec("core"),) * n_outs
    donate = tuple(range(n_params, n_params + n_outs))
    sharded = jax.jit(
        shard_map(
            _body, mesh=mesh, in_specs=in_specs, out_specs=out_specs, check_rep=False
        ),
        donate_argnums=donate,
        keep_unused=True,
    )
    _CACHE[key] = (sharded, in_names, out_names, out_avals, zero_outs, mesh)
    return _CACHE[key]


def _concat_inputs(in_maps, bench_iters=1):
    sharded, in_names, out_names, out_avals, zero_outs, mesh = _get_runner(bench_iters)
    concat_in = [
        np.concatenate([np.asarray(m[n]) for m in in_maps], axis=0) for n in in_names
    ]
    concat_zeros = [
        np.zeros((B * z.shape[0], *z.shape[1:]), z.dtype) for z in zero_outs
    ]
    return concat_in, concat_zeros


def _run_spmd(in_maps):
    sharded, in_names, out_names, out_avals, zero_outs, mesh = _get_runner()
    concat_in, concat_zeros = _concat_inputs(in_maps)
    out_arrs = sharded(*concat_in, *concat_zeros)
    return [
        {
            name: np.asarray(out_arrs[i]).reshape(B, *out_avals[i].shape)[c]
            for i, name in enumerate(out_names)
        }
        for c in range(B)
    ]


def _wall_times(in_maps, bench_iters, reps):
    import time

    import jax
    from jax.sharding import NamedSharding, PartitionSpec

    sharded, in_names, out_names, out_avals, zero_outs, mesh = _get_runner(bench_iters)
    concat_in, concat_zeros = _concat_inputs(in_maps, bench_iters)
    sh = NamedSharding(mesh, PartitionSpec("core"))
    dev_in = [jax.device_put(a, sh) for a in concat_in]
    jax.block_until_ready(dev_in)
    times = []
    out = None
    for _ in range(reps + 1):
        dev_z = [jax.device_put(a, sh) for a in concat_zeros]
        jax.block_until_ready(dev_z)
        t0 = time.perf_counter()
        out = sharded(*dev_in, *dev_z)
        jax.block_until_ready(out)
        times.append((time.perf_counter() - t0) * 1e9)
    return times[1:], out


def bench(in_maps, lo=129, hi=2049, reps=7):
    """Device-side loop timing: the kernel body repeats inside one NEFF via
    For_i; per-iteration time is the slope between two large loop counts,
    which cancels the (noisy) axon RPC overhead."""
    import time

    import jax
    from jax.sharding import NamedSharding, PartitionSpec

    runs = {}
    for it in (lo, hi):
        sharded, in_names, out_names, out_avals, zero_outs, mesh = _get_runner(it)
        ci, cz = _concat_inputs(in_maps, it)
        sh = NamedSharding(mesh, PartitionSpec("core"))
        dev_in = [jax.device_put(a, sh) for a in ci]
        jax.block_until_ready(dev_in)
        runs[it] = (sharded, dev_in, cz, sh)
    times = {lo: [], hi: []}
    for r in range(reps + 1):
        for it in (lo, hi):
            sharded, dev_in, cz, sh = runs[it]
            dev_z = [jax.device_put(a, sh) for a in cz]
            jax.block_until_ready(dev_z)
            t0 = time.perf_counter()
            out = sharded(*dev_in, *dev_z)
            jax.block_until_ready(out)
            if r > 0:
                times[it].append(time.perf_counter() - t0)
    per_iter = (min(times[hi]) - min(times[lo])) / (hi - lo) * 1e9
    return per_iter, (min(times[lo]) * 1e9, min(times[hi]) * 1e9)


def kernel(**inputs):
    results = _run_spmd(make_in_maps(inputs))
    out = np.stack([r["out"].reshape(NT, NT, 2) for r in results])
    return out.astype(np.float32)
